# revision 16
# baseline (speedup 1.0000x reference)
"""Batched cosine-similarity matrix (retrieval_knn) on 8 TRN2 NeuronCores.

reference:  out[b, n, m] = <x[b,n,:], y[b,m,:]> / max(||x[b,n]|| * ||y[b,m]||, 1e-8)
shapes:     x, y: [8, 2048, 512] f32  ->  out: [8, 2048, 2048] f32

Sharding: data-parallel over the batch dim — batch b runs on core b.
Each core receives x[b].T and y[b].T cast to bf16 host-side (contraction
dim d on SBUF partitions; bf16 halves DMA traffic and enables the PE's
fast weight load, which fp32/f32r weights cannot use).

Per-core kernel (all-bf16 matmul pipeline, f32 PSUM accumulation):
  xsq/ysq = x*x, y*y                (DVE, bf16 2x mode)
  snx/sny = ones.T @ sq             (PE, replicated column sums, f32 PSUM)
  rs      = 1/sqrt(s)               (ACT sqrt + DVE reciprocal_approx_fast
                                     + bf16 cast; ACT Rsqrt is banned)
  xhat    = x * rsnx                (DVE; folds the row-norm scale into
                                     the matmul weights so the epilogue
                                     only needs the column-norm factor)
  dots    = xhat.T @ y              (PE, 64 tiles [128,512], k=4x128)
  out     = dots * rsny             (DVE tensor_tensor, bf16 out)
All x-side norm work is front-loaded before the main loop (mid-loop norm
work convoys the in-order engine queues — measured).  Only the tiny
y-norm finish for block c+1 (4 PE matmuls + sqrt + reciprocal + cast,
~2us total) is hoisted into block c so block boundaries never stall on
it.  A burst of dummy matmuls on the constants tile warms the PE's HAM
activity window during the input DMA so the norm matmul chains run at
full clock instead of half.  Output is written bf16, upcast host-side.
"""

import sys

if "/opt/trn_rl_repo" not in sys.path:
    sys.path.insert(0, "/opt/trn_rl_repo")

import numpy as np
import ml_dtypes

import concourse.bass as bass
import concourse.bacc as bacc
import concourse.mybir as mybir
import concourse.tile as tile
from concourse.bass_utils import run_bass_kernel_spmd

P = 128          # partitions
D = 512          # feature dim (contraction)
N = 2048         # rows of x / y
B = 8            # batch == n_cores
KC = D // P      # 4 k-chunks
NT = N // P      # 16 n-tiles (output partition tiles)
MC = N // 512    # 4 m-chunks (output free chunks, PSUM-bank width)
WARMUP_MM = 28   # dummy [128,128] matmuls to warm the PE clock

F32 = mybir.dt.float32
BF16 = mybir.dt.bfloat16

_CACHED = {}


def _build_nc() -> bass.Bass:
    """Build the single-core Bass program (same program runs SPMD on 8 cores)."""
    nc = bacc.Bacc(trn_type="TRN2", target_bir_lowering=False, debug=False)

    xT = nc.dram_tensor("xT", [D, N], BF16, kind="ExternalInput").ap()
    yT = nc.dram_tensor("yT", [D, N], BF16, kind="ExternalInput").ap()
    out = nc.dram_tensor("out", [N, N], BF16, kind="ExternalOutput").ap()

    with tile.TileContext(nc) as tc:
        with (
            tc.tile_pool(name="xin", bufs=1) as xin_pool,
            tc.tile_pool(name="yin", bufs=1) as yin_pool,
            tc.tile_pool(name="sq", bufs=1) as sq_pool,
            tc.tile_pool(name="consts", bufs=1) as const_pool,
            tc.tile_pool(name="norms", bufs=1) as norm_pool,
            tc.tile_pool(name="xh", bufs=1) as xh_pool,
            tc.tile_pool(name="ostage", bufs=6) as out_pool,
            tc.tile_pool(name="mm_ps", bufs=6, space="PSUM") as mm_ps_pool,
            tc.tile_pool(name="n_ps", bufs=2, space="PSUM") as n_ps_pool,
        ):
            ones_f = const_pool.tile([P, P], F32, name="ones_f")
            nc.vector.memset(ones_f, 1.0)
            ones = const_pool.tile([P, P], BF16, name="ones")
            nc.scalar.copy(ones, ones_f)

            # PE warm-up: keep the HAM activity window busy while the first
            # input chunks stream in, so the norm matmul chains run at full
            # clock (cold matmuls take ~2.7x longer).
            wps = n_ps_pool.tile([P, 512], F32, name="wps", tag="n_ps")
            for _ in range(WARMUP_MM):
                nc.tensor.matmul(wps[:, 0:P], lhsT=ones, rhs=ones,
                                 start=True, stop=True)

            # ---- input DMAs ------------------------------------------
            # x first (the x-norm pipeline needs all of x before the first
            # main matmul group can be built); y in m-block-major order so
            # the first output column block unlocks as early as possible.
            xt, yt = [], []
            for k in range(KC):
                xk = xin_pool.tile([P, N], BF16, name=f"xt{k}", tag=f"xt{k}")
                nc.sync.dma_start(out=xk, in_=xT[k * P:(k + 1) * P, :])
                xt.append(xk)
            for k in range(KC):
                yk = yin_pool.tile([P, N], BF16, name=f"yt{k}", tag=f"yt{k}")
                yt.append(yk)
            for c in range(MC):
                cs = slice(c * 512, (c + 1) * 512)
                for k in range(KC):
                    nc.sync.dma_start(out=yt[k][:, cs], in_=yT[k * P:(k + 1) * P, cs])

            # ---- x norm pipeline (all front-loaded) ------------------
            xsq = []
            for k in range(KC):
                xs = sq_pool.tile([P, N], BF16, name=f"xsq{k}", tag=f"xsq{k}")
                nc.vector.tensor_tensor(xs, xt[k], xt[k], mybir.AluOpType.mult)
                xsq.append(xs)

            ysq = []
            for k in range(KC):
                ys = sq_pool.tile([P, N], BF16, name=f"ysq{k}", tag=f"ysq{k}")
                ysq.append(ys)

            snx = norm_pool.tile([P, N], F32, name="snx")
            rsnx_f = norm_pool.tile([P, N], F32, name="rsnx_f")
            rsnx = norm_pool.tile([P, N], BF16, name="rsnx")
            sny = norm_pool.tile([P, N], F32, name="sny")
            rsny_f = norm_pool.tile([P, N], F32, name="rsny_f")
            rsny = norm_pool.tile([P, N], BF16, name="rsny")

            xhat = [xh_pool.tile([P, N], BF16, name=f"xh{k}", tag=f"xh{k}")
                    for k in range(KC)]

            for c in range(MC):
                cs = slice(c * 512, (c + 1) * 512)
                # ysq for this m-block early so the PE's sny matmuls are
                # never stuck behind later DVE work.
                for k in range(KC):
                    nc.vector.tensor_tensor(ysq[k][:, cs], yt[k][:, cs],
                                            yt[k][:, cs], mybir.AluOpType.mult)
                n_ps = n_ps_pool.tile([P, 512], F32, name="n_ps", tag="n_ps")
                for k in range(KC):
                    nc.tensor.matmul(n_ps, lhsT=ones, rhs=xsq[k][:, cs],
                                     start=(k == 0), stop=(k == KC - 1))
                nc.scalar.sqrt(snx[:, cs], n_ps)
                nc.vector.reciprocal_approx_fast(rsnx_f[:, cs], snx[:, cs])
                nc.vector.tensor_copy(rsnx[:, cs], rsnx_f[:, cs])
                for k in range(KC):
                    nc.vector.tensor_tensor(xhat[k][:, cs], xt[k][:, cs],
                                            rsnx[:, cs], mybir.AluOpType.mult)

            # ---- y norm finish (replicated column sums -> 1/sqrt) ----
            def norm_y(c):
                cs = slice(c * 512, (c + 1) * 512)
                n_ps = n_ps_pool.tile([P, 512], F32, name="n_ps", tag="n_ps")
                for k in range(KC):
                    nc.tensor.matmul(n_ps, lhsT=ones, rhs=ysq[k][:, cs],
                                     start=(k == 0), stop=(k == KC - 1))
                nc.scalar.sqrt(sny[:, cs], n_ps)
                nc.vector.reciprocal_approx_fast(rsny_f[:, cs], sny[:, cs])
                nc.vector.tensor_copy(rsny[:, cs], rsny_f[:, cs])

            norm_y(0)

            # ---- main loop: m-block-major, 16 output tiles each ------
            for c in range(MC):
                cs = slice(c * 512, (c + 1) * 512)
                for t in range(NT):
                    if t == 6 and c + 1 < MC:
                        norm_y(c + 1)
                    ts_ = slice(t * P, (t + 1) * P)
                    ps = mm_ps_pool.tile([P, 512], F32, name="ps", tag="ps")
                    for k in range(KC):
                        nc.tensor.matmul(
                            ps, lhsT=xhat[k][:, ts_], rhs=yt[k][:, cs],
                            start=(k == 0), stop=(k == KC - 1),
                        )
                    ot = out_pool.tile([P, 512], BF16, name="ot", tag="ot")
                    nc.vector.tensor_tensor(ot, ps, rsny[:, cs],
                                            mybir.AluOpType.mult)
                    nc.sync.dma_start(out=out[ts_, cs], in_=ot)

    nc.compile()
    return nc


def _get_nc() -> bass.Bass:
    if "bf16" not in _CACHED:
        _CACHED["bf16"] = _build_nc()
    return _CACHED["bf16"]


def _shard(x: np.ndarray, y: np.ndarray):
    """Host-side sharding: batch b -> core b, transposed to [512, 2048] bf16."""
    x = np.asarray(x, dtype=np.float32)
    y = np.asarray(y, dtype=np.float32)
    xTs = np.ascontiguousarray(np.transpose(x, (0, 2, 1))).astype(ml_dtypes.bfloat16)
    yTs = np.ascontiguousarray(np.transpose(y, (0, 2, 1))).astype(ml_dtypes.bfloat16)
    return [{"xT": xTs[b], "yT": yTs[b]} for b in range(B)]


def _run(x: np.ndarray, y: np.ndarray, mm_dtype: str = "bf16",
         trace: bool = False):
    """Returns (out [8, 2048, 2048] f32, BassKernelResults)."""
    nc = _get_nc()
    in_maps = _shard(x, y)
    res = run_bass_kernel_spmd(nc, in_maps, core_ids=list(range(B)), trace=trace)
    out = np.stack([res.results[b]["out"].astype(np.float32) for b in range(B)])
    return out, res


def kernel(x: np.ndarray, y: np.ndarray) -> np.ndarray:
    out, _ = _run(x, y)
    return out
